# revision 1
# baseline (speedup 1.0000x reference)
"""Multi-head attention (b=2, n=2048, d_model=1024, h=16, d_k=d_v=64) + relu(fc) +
residual + LayerNorm, sharded over 8 NeuronCores.

Sharding: core i = (batch bi = i//4) x (head-group hg = i%4, 4 heads each).
Per core: QKV projections for its head group (bf16 matmuls), attention computed
in "S^T" layout (keys on partitions, queries on free axis) so no transposes are
needed; a ones-augmented value matrix folds the softmax denominator into the
context matmul; partial fc (its 256 rows of w_fc); a 4-rank ReduceScatter per
512-query slab (overlapped with the next attention tile) sums fc partials and
leaves each core 128 rows per slab, on which relu + residual run immediately
and LayerNorm at the end.  Host reassembles the 8 x [512, 1024] outputs.
"""

import numpy as np
import ml_dtypes
from contextlib import ExitStack

B = 2
N = 2048
D = 1024
H = 16
DK = 64
HL = H // 4          # heads per core
CSL = HL * DK        # 256 per-core fc contraction
ROWS = N // 4        # 512 output rows per core
LN_EPS = 1e-6
N_CORES = 8

_CACHE = {}


def _build():
    import concourse.bass as bass
    import concourse.tile as tile
    import concourse.mybir as mybir
    from concourse import bacc

    bf16 = mybir.dt.bfloat16
    f32 = mybir.dt.float32
    AF = mybir.ActivationFunctionType
    Alu = mybir.AluOpType

    nc = bacc.Bacc("TRN2", target_bir_lowering=False, debug=False,
                   num_devices=N_CORES)

    qT = nc.dram_tensor("qT", [D, N], bf16, kind="ExternalInput").ap()
    kT = nc.dram_tensor("kT", [D, N], bf16, kind="ExternalInput").ap()
    vT = nc.dram_tensor("vT", [D, N], bf16, kind="ExternalInput").ap()
    wq = nc.dram_tensor("wq", [D, CSL], bf16, kind="ExternalInput").ap()
    wk = nc.dram_tensor("wk", [D, CSL], bf16, kind="ExternalInput").ap()
    wv = nc.dram_tensor("wv", [D, CSL], bf16, kind="ExternalInput").ap()
    wfc = nc.dram_tensor("wfc", [CSL, D], bf16, kind="ExternalInput").ap()
    qres = nc.dram_tensor("qres", [ROWS, D], f32, kind="ExternalInput").ap()
    gamma = nc.dram_tensor("gamma", [D], f32, kind="ExternalInput").ap()
    beta = nc.dram_tensor("beta", [D], f32, kind="ExternalInput").ap()
    y = nc.dram_tensor("y", [ROWS, D], f32, kind="ExternalOutput").ap()

    KC = D // 128     # 8 contraction chunks for projections
    ST = N // 512     # 4 seq tiles of 512 queries
    SC = N // 128     # 16 seq chunks of 128 keys
    G = 2             # key chunks per exp batch
    NG = SC // G

    with tile.TileContext(nc) as tc:
        with ExitStack() as ctx:
            persist = ctx.enter_context(tc.tile_pool(name="persist", bufs=1))
            work = ctx.enter_context(tc.tile_pool(name="work", bufs=2))
            epool = ctx.enter_context(tc.tile_pool(name="epool", bufs=4))
            pat = ctx.enter_context(tc.tile_pool(name="pat", bufs=1, space="PSUM"))
            dram = ctx.enter_context(tc.tile_pool(name="dram", bufs=2, space="DRAM"))
            late_ctx = ExitStack()
            late = late_ctx.enter_context(tc.tile_pool(name="late", bufs=1))
            qkv_ctx = ExitStack()
            qkv = qkv_ctx.enter_context(tc.tile_pool(name="qkv", bufs=1))

            # PSUM tags: "s" scores [128,2,512] x3 = 6 banks, "c" context
            # [65,512] x2 = 2 banks.  Projection/fc psums ([128,<=512]) borrow
            # "s" slots (they fit in the 2-bank slot and never run at the same
            # time as a full score pipeline).
            def ps_s():
                return pat.tile([128, G, 512], f32, tag="s", name="ps_s", bufs=3)

            def ps_c():
                return pat.tile([DK + 1, 512], f32, tag="c", name="ps_c", bufs=2)

            def ps_f(n=512):
                return pat.tile([128, n], f32, tag="s", name="ps_f", bufs=3)

            # ---- load inputs -------------------------------------------------
            qT_sb = qkv.tile([128, KC, N], bf16, tag="qT", name="qT")
            kT_sb = qkv.tile([128, KC, N], bf16, tag="kT", name="kT")
            vT_sb = qkv.tile([128, KC, N], bf16, tag="vT", name="vT")
            for kc in range(KC):
                nc.sync.dma_start(out=qT_sb[:, kc, :], in_=qT[kc * 128:(kc + 1) * 128, :])
                nc.sync.dma_start(out=kT_sb[:, kc, :], in_=kT[kc * 128:(kc + 1) * 128, :])
                nc.sync.dma_start(out=vT_sb[:, kc, :], in_=vT[kc * 128:(kc + 1) * 128, :])
            wq_sb = qkv.tile([128, KC, CSL], bf16, tag="wq", name="wq")
            wk_sb = qkv.tile([128, KC, CSL], bf16, tag="wk", name="wk")
            wv_sb = qkv.tile([128, KC, CSL], bf16, tag="wv", name="wv")
            nc.sync.dma_start(out=wq_sb, in_=wq.rearrange("(c p) m -> p c m", p=128))
            nc.sync.dma_start(out=wk_sb, in_=wk.rearrange("(c p) m -> p c m", p=128))
            nc.sync.dma_start(out=wv_sb, in_=wv.rearrange("(c p) m -> p c m", p=128))

            # ---- projections -------------------------------------------------
            # qhT/khT: [dk, seq] per head, two heads stacked per 128-partition
            # tile (head 2p on partitions 0-63, head 2p+1 on 64-127).
            qhT = [persist.tile([128, N], bf16, tag=f"qhT{p}", name=f"qhT{p}") for p in range(2)]
            khT = [persist.tile([128, N], bf16, tag=f"khT{p}", name=f"khT{p}") for p in range(2)]
            vh = [persist.tile([128, HL, DK + 1], bf16, tag=f"vh{sc}", name=f"vh{sc}") for sc in range(SC)]

            def qk_proj(p):
                for st in range(ST):
                    for dst, w_sb, src in ((qhT, wq_sb, qT_sb), (khT, wk_sb, kT_sb)):
                        ps = ps_f()
                        for kc in range(KC):
                            nc.tensor.matmul(
                                ps,
                                w_sb[:, kc, p * 128:(p + 1) * 128],
                                src[:, kc, st * 512:(st + 1) * 512],
                                start=(kc == 0), stop=(kc == KC - 1))
                        nc.vector.tensor_copy(out=dst[p][:, st * 512:(st + 1) * 512], in_=ps)

            for sc in range(SC):
                nc.vector.memset(vh[sc][:, :, DK:DK + 1], 1.0)

            def v_proj(sc0, sc1):
                for sc in range(sc0, sc1):
                    ps = ps_f(CSL)
                    for kc in range(KC):
                        nc.tensor.matmul(
                            ps,
                            vT_sb[:, kc, sc * 128:(sc + 1) * 128],
                            wv_sb[:, kc, :],
                            start=(kc == 0), stop=(kc == KC - 1))
                    nc.vector.tensor_copy(
                        out=vh[sc][:, :, 0:DK],
                        in_=ps.rearrange("p (h d) -> p h d", h=HL))

            # ctxn: normalized context, transposed: [c, seq]; two heads stacked.
            ctxn = [persist.tile([128, N], bf16, tag=f"ctxn{p}", name=f"ctxn{p}") for p in range(2)]

            def attention(p, t, extra=None):
                # heads 2p (partitions 0-63) and 2p+1 (64-127) interleaved so
                # their score matmuls run concurrently in distinct row groups.
                pc = [ps_c() for _ in range(2)]
                ppss = [None, None]
                pse = [None, None]
                for g in range(NG):
                    if extra is not None:
                        extra(g)
                    for s in range(2):
                        lo, hi = 64 * s, 64 * (s + 1)
                        ppss[s] = ps_s()
                        for j in range(G):
                            kc = g * G + j
                            nc.tensor.matmul(
                                ppss[s][:, j, :],
                                khT[p][lo:hi, kc * 128:(kc + 1) * 128],
                                qhT[p][lo:hi, t * 512:(t + 1) * 512],
                                start=True, stop=True)
                    for s in range(2):
                        pse[s] = epool.tile([128, G, 512], bf16, tag="e", name="e")
                        nc.scalar.activation(out=pse[s], in_=ppss[s], func=AF.Exp,
                                             scale=1.0 / float(np.sqrt(DK)))
                    for s in range(2):
                        for j in range(G):
                            kc = g * G + j
                            nc.tensor.matmul(
                                pc[s],
                                vh[kc][:, 2 * p + s, :],
                                pse[s][:, j, :],
                                start=(kc == 0), stop=(kc == SC - 1))
                for s in range(2):
                    rb = work.tile([DK, 512], f32, tag="rb", name="rb")
                    nc.vector.reciprocal(out=rb[0:1, :], in_=pc[s][DK:DK + 1, :])
                    cun = work.tile([DK, 512], f32, tag="cun", name="cun")
                    nc.vector.tensor_copy(out=cun, in_=pc[s][0:DK, :])
                    r_dram = dram.tile([1, 512], f32, tag="rd", name="rd")
                    nc.gpsimd.dma_start(out=r_dram, in_=rb[0:1, :])
                    nc.gpsimd.dma_start(
                        out=rb,
                        in_=bass.AP(tensor=r_dram.tensor, offset=r_dram.offset,
                                    ap=[[0, DK]] + r_dram.ap[1:]))
                    nc.vector.tensor_mul(
                        out=ctxn[p][64 * s:64 * (s + 1), t * 512:(t + 1) * 512],
                        in0=cun, in1=rb)

            # fc + chunked ReduceScatter per 512-query slab t; each core ends
            # with rows [t*512 + rank*128, +128) of its batch.
            wfc_sb = late.tile([128, CSL // 128, D], bf16, tag="wfc", name="wfc")
            nc.sync.dma_start(out=wfc_sb, in_=wfc.rearrange("(c p) n -> p c n", p=128))
            qres_sb = late.tile([128, ST, D], f32, tag="qres", name="qres")
            nc.sync.dma_start(out=qres_sb, in_=qres.rearrange("(c p) n -> p c n", p=128))
            xacc = qres_sb  # relu+residual accumulates in place over the residual

            def fc_rs(t):
                rs_in = dram.tile([512, D], bf16, tag="rs_in", name="rs_in")
                rs_out = dram.tile([128, D], bf16, tag="rs_out", name="rs_out")
                for qq in range(4):
                    qc = t * 4 + qq
                    for nh in range(2):
                        ps = ps_f()
                        for cc in range(CSL // 128):
                            nc.tensor.matmul(
                                ps,
                                ctxn[cc][:, qc * 128:(qc + 1) * 128],
                                wfc_sb[:, cc, nh * 512:(nh + 1) * 512],
                                start=(cc == 0), stop=(cc == CSL // 128 - 1))
                        fcs = work.tile([128, 512], bf16, tag="fcs", name="fcs")
                        nc.vector.tensor_copy(out=fcs, in_=ps)
                        nc.sync.dma_start(
                            out=rs_in[qq * 128:(qq + 1) * 128, nh * 512:(nh + 1) * 512],
                            in_=fcs)
                nc.gpsimd.collective_compute(
                    "ReduceScatter",
                    mybir.AluOpType.add,
                    replica_groups=[[0, 1, 2, 3], [4, 5, 6, 7]],
                    ins=[rs_in.opt()],
                    outs=[rs_out.opt()])
                rs_sb = work.tile([128, D], bf16, tag="rs_sb", name="rs_sb")
                nc.sync.dma_start(out=rs_sb, in_=rs_out)
                # relu (DVE) + residual now; LayerNorm deferred to the end.
                xr = work.tile([128, D], f32, tag="xr", name="xr")
                nc.vector.tensor_scalar(out=xr, in0=rs_sb,
                                        scalar1=0.0, scalar2=None, op0=Alu.max)
                nc.vector.tensor_add(out=xacc[:, t, :], in0=xr,
                                     in1=qres_sb[:, t, :])

            qk_proj(0)
            # v projection interleaved into the first attention tile: group g
            # of attention(0,0) consumes key chunks 2g/2g+1, whose vh tiles are
            # produced by the extra() emitted at group g+1 boundary... vh[kc]
            # for g's ctx must exist before that ctx matmul, so emit chunks
            # two groups ahead.
            v_proj(0, 4)
            attention(0, 0, extra=lambda g: v_proj(4 + 2 * g, min(4 + 2 * g + 2, SC)))
            qk_proj(1)
            attention(1, 0)
            for t in range(1, ST):
                attention(0, t)
                fc_rs(t - 1)
                attention(1, t)
            fc_rs(ST - 1)
            qkv_ctx.close()

            # ---- layernorm ---------------------------------------------------
            gamma_sb = late.tile([128, D], f32, tag="gamma", name="gamma")
            nc.sync.dma_start(out=gamma_sb,
                              in_=bass.AP(tensor=gamma.tensor, offset=gamma.offset,
                                          ap=[[0, 128]] + gamma.ap))
            beta_sb = late.tile([128, D], f32, tag="beta", name="beta")
            nc.sync.dma_start(out=beta_sb,
                              in_=bass.AP(tensor=beta.tensor, offset=beta.offset,
                                          ap=[[0, 128]] + beta.ap))
            eps_sb = late.tile([128, 1], f32, tag="eps", name="eps")
            nc.vector.memset(eps_sb, LN_EPS)

            for t in range(ST):
                x = xacc[:, t, :]
                stats = work.tile([128, 2, 6], f32, tag="stats", name="stats")
                nc.vector.bn_stats(out=stats[:, 0, :], in_=x[:, 0:512])
                nc.vector.bn_stats(out=stats[:, 1, :], in_=x[:, 512:1024])
                mv = work.tile([128, 2], f32, tag="mv", name="mv")
                nc.vector.bn_aggr(out=mv, in_=stats)
                nc.scalar.activation(out=mv[:, 1:2], in_=mv[:, 1:2], func=AF.Sqrt,
                                     bias=eps_sb, scale=1.0)
                nc.vector.reciprocal(out=mv[:, 1:2], in_=mv[:, 1:2])
                xo = work.tile([128, D], f32, tag="xo", name="xo")
                nc.vector.tensor_scalar(out=xo, in0=x,
                                        scalar1=mv[:, 0:1], scalar2=mv[:, 1:2],
                                        op0=Alu.subtract, op1=Alu.mult)
                nc.vector.tensor_mul(out=xo, in0=xo, in1=gamma_sb)
                nc.vector.tensor_add(out=xo, in0=xo, in1=beta_sb)
                nc.sync.dma_start(out=y[t * 128:(t + 1) * 128, :], in_=xo)
            late_ctx.close()

    nc.compile()
    return nc


def kernel(q, k, v, w_qs, w_ks, w_vs, w_fc, ln_gamma, ln_beta):
    from concourse import bass_utils

    if "nc" not in _CACHE:
        _CACHE["nc"] = _build()
    nc = _CACHE["nc"]

    bf = ml_dtypes.bfloat16
    q = np.asarray(q, np.float32)
    k = np.asarray(k, np.float32)
    v = np.asarray(v, np.float32)
    w_fc = np.asarray(w_fc, np.float32)

    in_maps = []
    for i in range(N_CORES):
        bi, hg = i // 4, i % 4
        cs = slice(hg * CSL, (hg + 1) * CSL)
        # rows handled by this core: for each slab t, rows t*512+hg*128 .. +128
        row_idx = np.concatenate(
            [np.arange(t * 512 + hg * 128, t * 512 + (hg + 1) * 128) for t in range(4)])
        in_maps.append({
            "qT": np.ascontiguousarray(q[bi].T).astype(bf),
            "kT": np.ascontiguousarray(k[bi].T).astype(bf),
            "vT": np.ascontiguousarray(v[bi].T).astype(bf),
            "wq": np.ascontiguousarray(np.asarray(w_qs, np.float32)[:, cs]).astype(bf),
            "wk": np.ascontiguousarray(np.asarray(w_ks, np.float32)[:, cs]).astype(bf),
            "wv": np.ascontiguousarray(np.asarray(w_vs, np.float32)[:, cs]).astype(bf),
            "wfc": np.ascontiguousarray(w_fc[cs, :]).astype(bf),
            "qres": np.ascontiguousarray(q[bi][row_idx]),
            "gamma": np.ascontiguousarray(np.asarray(ln_gamma, np.float32)),
            "beta": np.ascontiguousarray(np.asarray(ln_beta, np.float32)),
        })

    run_kwargs = dict(_CACHE.get("run_kwargs", {}))
    res = bass_utils.run_bass_kernel_spmd(nc, in_maps, core_ids=list(range(N_CORES)),
                                          **run_kwargs)
    _CACHE["last_res"] = res
    out = np.empty((B, N, D), np.float32)
    for i in range(N_CORES):
        bi, hg = i // 4, i % 4
        yi = res.results[i]["y"]
        for t in range(4):
            out[bi, t * 512 + hg * 128:t * 512 + (hg + 1) * 128, :] = \
                yi[t * 128:(t + 1) * 128, :]
    return out



# revision 2
# speedup vs baseline: 1.0649x; 1.0649x over previous
"""Multi-head attention (b=2, n=2048, d_model=1024, h=16, d_k=d_v=64) + relu(fc) +
residual + LayerNorm, sharded over 8 NeuronCores.

Sharding: core i = (batch bi = i//4) x (head-group hg = i%4, 4 heads each).

v2 changes vs baseline:
- input DMAs split per 512-seq slab and ordered weights-first so the first
  projection matmul starts ~2us in instead of ~50us.
- score matmuls alternate PE row groups (heads 2p / 2p+1) every instruction so
  LDWEIGHTS overlaps the previous matmul (measured 216ns vs 336ns per MM).
- context matmuls run in fp8 DoubleRow mode: 256-key contraction per MM
  (pse + vh quantized to e4m3; ones column folds the softmax denominator in).
- fc matmuls run in fp8 DoubleRow (256-dim contraction per MM); w_fc is
  pre-scaled x64 on the host to dodge e4m3 subnormals, descaled at psum copy.
- software pipelining: per group g the PE stream is scores(g), ctx(g-1), with
  v_proj / qk_proj(1) / fc matmuls as fillers, because exp on the scalar
  engine (18.4us/tile) outpaces the attention matmuls (12.5us/tile).
"""

import numpy as np
import ml_dtypes
from contextlib import ExitStack

B = 2
N = 2048
D = 1024
H = 16
DK = 64
HL = H // 4          # heads per core
CSL = HL * DK        # 256 per-core fc contraction
ROWS = N // 4        # 512 output rows per core
LN_EPS = 1e-6
N_CORES = 8
WFC_SCALE = 64.0

_CACHE = {}


def _build():
    import concourse.bass as bass
    import concourse.tile as tile
    import concourse.mybir as mybir
    from concourse import bacc

    bf16 = mybir.dt.bfloat16
    f32 = mybir.dt.float32
    fp8 = mybir.dt.float8e4
    AF = mybir.ActivationFunctionType
    Alu = mybir.AluOpType
    DR = mybir.MatmulPerfMode.DoubleRow

    nc = bacc.Bacc("TRN2", target_bir_lowering=False, debug=False,
                   num_devices=N_CORES)

    qT = nc.dram_tensor("qT", [D, N], bf16, kind="ExternalInput").ap()
    kT = nc.dram_tensor("kT", [D, N], bf16, kind="ExternalInput").ap()
    vT = nc.dram_tensor("vT", [D, N], bf16, kind="ExternalInput").ap()
    wq = nc.dram_tensor("wq", [D, CSL], bf16, kind="ExternalInput").ap()
    wk = nc.dram_tensor("wk", [D, CSL], bf16, kind="ExternalInput").ap()
    wv = nc.dram_tensor("wv", [D, CSL], bf16, kind="ExternalInput").ap()
    wfc8 = nc.dram_tensor("wfc8", [CSL, D], fp8, kind="ExternalInput").ap()
    qres = nc.dram_tensor("qres", [ROWS, D], f32, kind="ExternalInput").ap()
    gamma = nc.dram_tensor("gamma", [D], f32, kind="ExternalInput").ap()
    beta = nc.dram_tensor("beta", [D], f32, kind="ExternalInput").ap()
    y = nc.dram_tensor("y", [ROWS, D], f32, kind="ExternalOutput").ap()

    KC = D // 128     # 8 contraction chunks for projections
    ST = N // 512     # 4 seq tiles of 512 queries
    SC = N // 128     # 16 seq chunks of 128 keys
    G = 2             # key chunks per exp batch / DR pair
    NG = SC // G      # 8 groups per attention tile
    FAST_EXP_GROUPS = set()   # gpsimd cast measured 4x too slow; keep exp on ACT
    EXP_A = float(2.0 ** 23) * float(np.log2(np.e)) / float(np.sqrt(DK))
    EXP_B = float(2.0 ** 23) * (127.0 - 0.04367)

    with tile.TileContext(nc) as tc:
        with ExitStack() as ctx:
            persist = ctx.enter_context(tc.tile_pool(name="persist", bufs=1))
            work = ctx.enter_context(tc.tile_pool(name="work", bufs=2))
            epool = ctx.enter_context(tc.tile_pool(name="epool", bufs=4))
            pat = ctx.enter_context(tc.tile_pool(name="pat", bufs=1, space="PSUM"))
            dram = ctx.enter_context(tc.tile_pool(name="dram", bufs=2, space="DRAM"))
            late_ctx = ExitStack()
            late = late_ctx.enter_context(tc.tile_pool(name="late", bufs=1))
            qkv_ctx = ExitStack()
            qkv = qkv_ctx.enter_context(tc.tile_pool(name="qkv", bufs=1))

            # PSUM: "s" [128,2,512] x3 = 6 banks (scores; proj/fc borrow these
            # slots), "c" [65,512] x2 = 2 banks (ctx accumulators).
            def ps_s():
                return pat.tile([128, G, 512], f32, tag="s", name="ps_s", bufs=3)

            def ps_c():
                return pat.tile([DK + 1, 512], f32, tag="c", name="ps_c", bufs=2)

            def ps_f(n=512):
                return pat.tile([128, n], f32, tag="s", name="ps_f", bufs=3)

            # ---- input loads: weights first, then per-slab slices ------------
            wq_sb = qkv.tile([128, KC, CSL], bf16, tag="wq", name="wq")
            wk_sb = qkv.tile([128, KC, CSL], bf16, tag="wk", name="wk")
            wv_sb = qkv.tile([128, KC, CSL], bf16, tag="wv", name="wv")
            nc.sync.dma_start(out=wq_sb, in_=wq.rearrange("(c p) m -> p c m", p=128))
            nc.sync.dma_start(out=wk_sb, in_=wk.rearrange("(c p) m -> p c m", p=128))

            qT_sb = qkv.tile([128, KC, N], bf16, tag="qT", name="qT")
            kT_sb = qkv.tile([128, KC, N], bf16, tag="kT", name="kT")
            vT_sb = qkv.tile([128, KC, N], bf16, tag="vT", name="vT")
            for half in range(2):
                sl = slice(half * 1024, (half + 1) * 1024)
                for kc in range(KC):
                    nc.sync.dma_start(out=qT_sb[:, kc, sl],
                                      in_=qT[kc * 128:(kc + 1) * 128, sl])
                    nc.sync.dma_start(out=kT_sb[:, kc, sl],
                                      in_=kT[kc * 128:(kc + 1) * 128, sl])
            nc.sync.dma_start(out=wv_sb, in_=wv.rearrange("(c p) m -> p c m", p=128))
            for half in range(2):
                sl = slice(half * 1024, (half + 1) * 1024)
                for kc in range(KC):
                    nc.sync.dma_start(out=vT_sb[:, kc, sl],
                                      in_=vT[kc * 128:(kc + 1) * 128, sl])

            # late inputs on the scalar HWDGE ring so they don't delay the above
            wfc_sb = late.tile([128, CSL // 128, D], fp8, tag="wfc", name="wfc")
            nc.scalar.dma_start(out=wfc_sb,
                                in_=wfc8.rearrange("(c p) n -> p c n", p=128))
            qres_sb = late.tile([128, ST, D], f32, tag="qres", name="qres")
            nc.scalar.dma_start(out=qres_sb,
                                in_=qres.rearrange("(c p) n -> p c n", p=128))
            gamma_sb = late.tile([128, D], f32, tag="gamma", name="gamma")
            nc.scalar.dma_start(out=gamma_sb,
                                in_=bass.AP(tensor=gamma.tensor, offset=gamma.offset,
                                            ap=[[0, 128]] + gamma.ap))
            beta_sb = late.tile([128, D], f32, tag="beta", name="beta")
            nc.scalar.dma_start(out=beta_sb,
                                in_=bass.AP(tensor=beta.tensor, offset=beta.offset,
                                            ap=[[0, 128]] + beta.ap))
            xacc = qres_sb  # relu+residual accumulates in place over the residual

            # ---- projections -------------------------------------------------
            # qhT/khT: [dk, seq] per head, heads 2p / 2p+1 stacked on partition
            # halves.  k copies go through the scalar engine pre-attention so
            # DVE and ACT split the psum-evacuation work.
            qhT = [persist.tile([128, N], bf16, tag=f"qhT{p}", name=f"qhT{p}") for p in range(2)]
            khT = [persist.tile([128, N], bf16, tag=f"khT{p}", name=f"khT{p}") for p in range(2)]

            def qk_proj_pair(p, st, use_act=False):
                # q and k chains interleaved on two psum banks: consecutive
                # MMs never hit the same bank (WAW chain penalty) and LDW
                # overlaps the previous matmul.
                sl = slice(st * 512, (st + 1) * 512)
                psq = ps_f()
                psk = ps_f()
                for kc in range(KC):
                    nc.tensor.matmul(
                        psq, wq_sb[:, kc, p * 128:(p + 1) * 128],
                        qT_sb[:, kc, sl],
                        start=(kc == 0), stop=(kc == KC - 1))
                    nc.tensor.matmul(
                        psk, wk_sb[:, kc, p * 128:(p + 1) * 128],
                        kT_sb[:, kc, sl],
                        start=(kc == 0), stop=(kc == KC - 1))
                nc.vector.tensor_copy(out=qhT[p][:, sl], in_=psq)
                if use_act:
                    nc.scalar.copy(out=khT[p][:, sl], in_=psk)
                else:
                    nc.vector.tensor_copy(out=khT[p][:, sl], in_=psk)

            # vh pairs for DoubleRow ctx: [128 keys, 2 chunks, HL heads, 80]
            # (65 used: 64 dims + ones column; 80 keeps the DR pair stride
            # 16B-aligned).
            vhp = [persist.tile([128, G, HL, 80], fp8, tag=f"vhp{g}", name=f"vhp{g}")
                   for g in range(NG)]
            for g in range(NG):
                nc.vector.memset(vhp[g][:, :, :, DK:DK + 1], 1.0)

            def v_proj_pair(g):
                # chunks 2g / 2g+1 interleaved on two psum banks
                sc0 = 2 * g
                psa = ps_f(CSL)
                psb = ps_f(CSL)
                for kc in range(KC):
                    nc.tensor.matmul(
                        psa, vT_sb[:, kc, sc0 * 128:(sc0 + 1) * 128],
                        wv_sb[:, kc, :],
                        start=(kc == 0), stop=(kc == KC - 1))
                    nc.tensor.matmul(
                        psb, vT_sb[:, kc, (sc0 + 1) * 128:(sc0 + 2) * 128],
                        wv_sb[:, kc, :],
                        start=(kc == 0), stop=(kc == KC - 1))
                nc.vector.tensor_copy(
                    out=vhp[g][:, 0, :, 0:DK],
                    in_=psa.rearrange("p (h d) -> p h d", h=HL))
                nc.vector.tensor_copy(
                    out=vhp[g][:, 1, :, 0:DK],
                    in_=psb.rearrange("p (h d) -> p h d", h=HL))

            # ctxn: normalized context, fp8, [128 c, 2 cc-chunks, seq] --
            # cc chunk index == p (heads 2p, 2p+1 on partition halves).
            ctxn = persist.tile([128, 2, N], fp8, tag="ctxn", name="ctxn")
            ones_sb = persist.tile([1, DK], bf16, tag="ones", name="ones")
            nc.vector.memset(ones_sb, 1.0)

            def attention(p, t, extra=None):
                pc = [ps_c() for _ in range(2)]
                pse_prev = [None, None]

                def ctx_mm(g, s):
                    nc.tensor.matmul(
                        pc[s],
                        vhp[g][:, :, 2 * p + s, 0:DK + 1],
                        pse_prev[s],
                        start=(g == 0), stop=(g == NG - 1),
                        perf_mode=DR)

                for g in range(NG):
                    ppss = [ps_s(), ps_s()]
                    # scores: alternate row groups every MM so LDW overlaps
                    for j in range(G):
                        kc = g * G + j
                        for s in range(2):
                            lo = 64 * s
                            nc.tensor.matmul(
                                ppss[s][:, j, :],
                                khT[p][lo:lo + 64, kc * 128:(kc + 1) * 128],
                                qhT[p][lo:lo + 64, t * 512:(t + 1) * 512],
                                start=True, stop=True,
                                tile_position=(lo, 0))
                    # ctx for the previous group (exp ran during these scores)
                    if g > 0:
                        for s in range(2):
                            ctx_mm(g - 1, s)
                    if extra is not None:
                        extra(g)
                    for s in range(2):
                        pse = epool.tile([128, G, 512], fp8, tag="e", name="e")
                        if g in FAST_EXP_GROUPS:
                            # Schraudolph exp on DVE (bit-trick) + gpsimd
                            # bitcast-convert: offloads the ACT bottleneck.
                            u = epool.tile([128, G, 512], mybir.dt.int32,
                                           tag="u", name="u")
                            nc.vector.tensor_scalar(
                                out=u, in0=ppss[s], scalar1=EXP_A, scalar2=EXP_B,
                                op0=Alu.mult, op1=Alu.add)
                            nc.gpsimd.tensor_copy(out=pse, in_=u.bitcast(f32))
                        else:
                            nc.scalar.activation(out=pse, in_=ppss[s], func=AF.Exp,
                                                 scale=1.0 / float(np.sqrt(DK)))
                        pse_prev[s] = pse
                for s in range(2):
                    ctx_mm(NG - 1, s)

                # normalize: broadcast the ones-row (softmax denominator)
                # across 64 partitions with a K=1 ones-matmul on the PE, take
                # the fast approx reciprocal (1.2 cpe vs 6), multiply into the
                # fp8 ctxn tile.  No gpsimd / DRAM round trip on this path.
                for s in range(2):
                    d1 = work.tile([1, 512], bf16, tag="rb", name="rb")
                    nc.vector.tensor_copy(out=d1, in_=pc[s][DK:DK + 1, :])
                    ps_rb = ps_f()
                    nc.tensor.matmul(ps_rb[0:DK, :], ones_sb, d1,
                                     start=True, stop=True)
                    rb = work.tile([DK, 512], f32, tag="rb64", name="rb64")
                    nc.vector.reciprocal_approx_fast(out=rb, in_=ps_rb[0:DK, :])
                    nc.vector.tensor_mul(
                        out=ctxn[64 * s:64 * (s + 1), p, t * 512:(t + 1) * 512],
                        in0=pc[s][0:DK, :], in1=rb)

            # fc (fp8 DoubleRow, both 128-chunks contracted per MM) + chunked
            # ReduceScatter per slab; each core ends with rows
            # [t*512 + rank*128, +128) of its batch.
            def fc_mm(t, qq, nh):
                qc = t * 4 + qq
                ps = ps_f()
                nc.tensor.matmul(
                    ps,
                    ctxn[:, :, qc * 128:(qc + 1) * 128],
                    wfc_sb[:, :, nh * 512:(nh + 1) * 512],
                    start=True, stop=True, perf_mode=DR)
                fcs = work.tile([128, 512], bf16, tag="fcs", name="fcs", bufs=8)
                nc.vector.tensor_scalar(out=fcs, in0=ps,
                                        scalar1=1.0 / WFC_SCALE, scalar2=None,
                                        op0=Alu.mult)
                return fcs

            rs_outs = []

            def fc_rs(t, fcs_parts):
                rs_in = dram.tile([512, D], bf16, tag="rs_in", name="rs_in",
                                  bufs=4)
                rs_out = dram.tile([128, D], bf16, tag="rs_out", name="rs_out",
                                   bufs=4)
                for (qq, nh), fcs in fcs_parts.items():
                    nc.sync.dma_start(
                        out=rs_in[qq * 128:(qq + 1) * 128, nh * 512:(nh + 1) * 512],
                        in_=fcs)
                nc.gpsimd.collective_compute(
                    "ReduceScatter",
                    mybir.AluOpType.add,
                    replica_groups=[[0, 1, 2, 3], [4, 5, 6, 7]],
                    ins=[rs_in.opt()],
                    outs=[rs_out.opt()])
                rs_outs.append(rs_out)

            # ---- schedule ----------------------------------------------------
            for st in range(ST):
                qk_proj_pair(0, st, use_act=True)
            v_proj_pair(0)
            v_proj_pair(1)

            # A(0,0) group g consumes vhp[g]; emit vhp[g+2] at group g, then
            # start qk_proj(1) in the last two group slots.
            def extra00(g):
                if g + 2 < NG:
                    v_proj_pair(g + 2)
                elif g == 6:
                    qk_proj_pair(1, 0)
                elif g == 7:
                    qk_proj_pair(1, 1)
            attention(0, 0, extra=extra00)
            qk_proj_pair(1, 2)
            qk_proj_pair(1, 3)
            attention(1, 0)

            fcs_pending = {}

            def extra_fc(t_prev):
                def fill(g):
                    # 8 fc MMs spread over the first 8 groups
                    if g < 8:
                        qq, nh = g // 2, g % 2
                        fcs_pending[(qq, nh)] = fc_mm(t_prev, qq, nh)
                return fill

            for t in range(1, ST):
                attention(0, t, extra=extra_fc(t - 1))
                fc_rs(t - 1, fcs_pending)
                fcs_pending = {}
                attention(1, t)
            for qq in range(4):
                for nh in range(2):
                    fcs_pending[(qq, nh)] = fc_mm(ST - 1, qq, nh)
            fc_rs(ST - 1, fcs_pending)
            qkv_ctx.close()

            # ---- tail: rs loads + relu/residual + layernorm ------------------
            # rstd on the DVE (bit-trick seed + 2 Newton iterations) keeps the
            # scalar engine's ACT table on the exp set for the whole kernel.
            i32 = mybir.dt.int32
            MAGIC = float(0x5F3759DF)

            for t in range(ST):
                rs_sb = work.tile([128, D], bf16, tag="rs_sb", name="rs_sb")
                # corner-write gate: forces the scheduler to keep this load
                # (which waits on the collective) after the last attention
                # tile, so it can't hoist into mid-kernel engine FIFOs.
                nc.gpsimd.tensor_copy(out=rs_sb[0:1, 0:1],
                                      in_=ctxn[0:1, 1, N - 1:N])
                nc.gpsimd.dma_start(out=rs_sb, in_=rs_outs[t])
                nc.vector.scalar_tensor_tensor(
                    out=xacc[:, t, :], in0=rs_sb, scalar=0.0,
                    in1=qres_sb[:, t, :], op0=Alu.max, op1=Alu.add)
                x = xacc[:, t, :]
                stats = work.tile([128, 2, 6], f32, tag="stats", name="stats")
                nc.vector.bn_stats(out=stats[:, 0, :], in_=x[:, 0:512])
                nc.vector.bn_stats(out=stats[:, 1, :], in_=x[:, 512:1024])
                mv = work.tile([128, 2], f32, tag="mv", name="mv")
                nc.vector.bn_aggr(out=mv, in_=stats)
                v = work.tile([128, 1], f32, tag="veps", name="veps")
                nc.vector.tensor_scalar(out=v, in0=mv[:, 1:2], scalar1=LN_EPS,
                                        scalar2=None, op0=Alu.add)
                # rsqrt: y0 = bitcast(0x5F3759DF - bits(v)/2) via f32 arithmetic
                # on the bit pattern (exact enough for a seed), then 2x Newton.
                si = work.tile([128, 1], i32, tag="rss", name="rss")
                nc.vector.tensor_scalar(out=si, in0=v.bitcast(i32), scalar1=-0.5,
                                        scalar2=MAGIC, op0=Alu.mult, op1=Alu.add)
                yk = si.bitcast(f32)
                for _ in range(2):
                    a = work.tile([128, 1], f32, tag="rsa", name="rsa")
                    nc.vector.tensor_mul(out=a, in0=yk, in1=yk)
                    nc.vector.tensor_mul(out=a, in0=a, in1=v)
                    nc.vector.tensor_scalar(out=a, in0=a, scalar1=-0.5,
                                            scalar2=1.5, op0=Alu.mult, op1=Alu.add)
                    yn = work.tile([128, 1], f32, tag="rsy", name="rsy")
                    nc.vector.tensor_mul(out=yn, in0=yk, in1=a)
                    yk = yn
                xo = work.tile([128, D], f32, tag="xo", name="xo")
                nc.vector.tensor_scalar(out=xo, in0=x,
                                        scalar1=mv[:, 0:1], scalar2=yk,
                                        op0=Alu.subtract, op1=Alu.mult)
                nc.vector.tensor_mul(out=xo, in0=xo, in1=gamma_sb)
                nc.vector.tensor_add(out=xo, in0=xo, in1=beta_sb)
                nc.sync.dma_start(out=y[t * 128:(t + 1) * 128, :], in_=xo)
            late_ctx.close()

    nc.compile()
    return nc


def kernel(q, k, v, w_qs, w_ks, w_vs, w_fc, ln_gamma, ln_beta):
    from concourse import bass_utils

    if "nc" not in _CACHE:
        _CACHE["nc"] = _build()
    nc = _CACHE["nc"]

    bf = ml_dtypes.bfloat16
    f8 = ml_dtypes.float8_e4m3
    q = np.asarray(q, np.float32)
    k = np.asarray(k, np.float32)
    v = np.asarray(v, np.float32)
    w_fc = np.asarray(w_fc, np.float32)

    in_maps = []
    for i in range(N_CORES):
        bi, hg = i // 4, i % 4
        cs = slice(hg * CSL, (hg + 1) * CSL)
        row_idx = np.concatenate(
            [np.arange(t * 512 + hg * 128, t * 512 + (hg + 1) * 128) for t in range(4)])
        in_maps.append({
            "qT": np.ascontiguousarray(q[bi].T).astype(bf),
            "kT": np.ascontiguousarray(k[bi].T).astype(bf),
            "vT": np.ascontiguousarray(v[bi].T).astype(bf),
            "wq": np.ascontiguousarray(np.asarray(w_qs, np.float32)[:, cs]).astype(bf),
            "wk": np.ascontiguousarray(np.asarray(w_ks, np.float32)[:, cs]).astype(bf),
            "wv": np.ascontiguousarray(np.asarray(w_vs, np.float32)[:, cs]).astype(bf),
            "wfc8": np.ascontiguousarray(w_fc[cs, :] * WFC_SCALE).astype(f8),
            "qres": np.ascontiguousarray(q[bi][row_idx]),
            "gamma": np.ascontiguousarray(np.asarray(ln_gamma, np.float32)),
            "beta": np.ascontiguousarray(np.asarray(ln_beta, np.float32)),
        })

    run_kwargs = dict(_CACHE.get("run_kwargs", {}))
    res = bass_utils.run_bass_kernel_spmd(nc, in_maps, core_ids=list(range(N_CORES)),
                                          **run_kwargs)
    _CACHE["last_res"] = res
    out = np.empty((B, N, D), np.float32)
    for i in range(N_CORES):
        bi, hg = i // 4, i % 4
        yi = res.results[i]["y"]
        for t in range(4):
            out[bi, t * 512 + hg * 128:t * 512 + (hg + 1) * 128, :] = \
                yi[t * 128:(t + 1) * 128, :]
    return out


# revision 3
# speedup vs baseline: 1.0843x; 1.0182x over previous
"""Multi-head attention (b=2, n=2048, d_model=1024, h=16, d_k=d_v=64) + relu(fc) +
residual + LayerNorm, sharded over 8 NeuronCores.

Sharding: core i = (batch bi = i//4) x (head-group hg = i%4, 4 heads each).

v2 changes vs baseline:
- input DMAs split per 512-seq slab and ordered weights-first so the first
  projection matmul starts ~2us in instead of ~50us.
- score matmuls alternate PE row groups (heads 2p / 2p+1) every instruction so
  LDWEIGHTS overlaps the previous matmul (measured 216ns vs 336ns per MM).
- context matmuls run in fp8 DoubleRow mode: 256-key contraction per MM
  (pse + vh quantized to e4m3; ones column folds the softmax denominator in).
- fc matmuls run in fp8 DoubleRow (256-dim contraction per MM); w_fc is
  pre-scaled x64 on the host to dodge e4m3 subnormals, descaled at psum copy.
- software pipelining: per group g the PE stream is scores(g), ctx(g-1), with
  v_proj / qk_proj(1) / fc matmuls as fillers, because exp on the scalar
  engine (18.4us/tile) outpaces the attention matmuls (12.5us/tile).
"""

import numpy as np
import ml_dtypes
from contextlib import ExitStack

B = 2
N = 2048
D = 1024
H = 16
DK = 64
HL = H // 4          # heads per core
CSL = HL * DK        # 256 per-core fc contraction
ROWS = N // 4        # 512 output rows per core
LN_EPS = 1e-6
N_CORES = 8
WFC_SCALE = 64.0

_CACHE = {}


def _build():
    import concourse.bass as bass
    import concourse.tile as tile
    import concourse.mybir as mybir
    from concourse import bacc

    bf16 = mybir.dt.bfloat16
    f32 = mybir.dt.float32
    fp8 = mybir.dt.float8e4
    AF = mybir.ActivationFunctionType
    Alu = mybir.AluOpType
    DR = mybir.MatmulPerfMode.DoubleRow

    nc = bacc.Bacc("TRN2", target_bir_lowering=False, debug=False,
                   num_devices=N_CORES)

    KCl = D // 128
    # weights/residual arrive pre-arranged from the host in SBUF layout so
    # their DMAs are contiguous 2-4KB-per-partition line-rate transfers
    qT = nc.dram_tensor("qT", [D, N], bf16, kind="ExternalInput").ap()
    kT = nc.dram_tensor("kT", [D, N], bf16, kind="ExternalInput").ap()
    vT = nc.dram_tensor("vT", [D, N], bf16, kind="ExternalInput").ap()
    wq = nc.dram_tensor("wq", [128, KCl, CSL], bf16, kind="ExternalInput").ap()
    wk = nc.dram_tensor("wk", [128, KCl, CSL], bf16, kind="ExternalInput").ap()
    wv = nc.dram_tensor("wv", [128, KCl, CSL], bf16, kind="ExternalInput").ap()
    wfc8 = nc.dram_tensor("wfc8", [128, CSL // 128, D], fp8, kind="ExternalInput").ap()
    qres = nc.dram_tensor("qres", [128, N // 512, D], f32, kind="ExternalInput").ap()
    gamma = nc.dram_tensor("gamma", [D], f32, kind="ExternalInput").ap()
    beta = nc.dram_tensor("beta", [D], f32, kind="ExternalInput").ap()
    y = nc.dram_tensor("y", [ROWS, D], f32, kind="ExternalOutput").ap()

    KC = D // 128     # 8 contraction chunks for projections
    ST = N // 512     # 4 seq tiles of 512 queries
    SC = N // 128     # 16 seq chunks of 128 keys
    G = 2             # key chunks per exp batch / DR pair
    NG = SC // G      # 8 groups per attention tile
    FAST_EXP_GROUPS = set()   # gpsimd cast measured 4x too slow; keep exp on ACT
    EXP_A = float(2.0 ** 23) * float(np.log2(np.e)) / float(np.sqrt(DK))
    EXP_B = float(2.0 ** 23) * (127.0 - 0.04367)

    with tile.TileContext(nc) as tc:
        with ExitStack() as ctx:
            persist = ctx.enter_context(tc.tile_pool(name="persist", bufs=1))
            work = ctx.enter_context(tc.tile_pool(name="work", bufs=2))
            epool = ctx.enter_context(tc.tile_pool(name="epool", bufs=4))
            pat = ctx.enter_context(tc.tile_pool(name="pat", bufs=1, space="PSUM"))
            dram = ctx.enter_context(tc.tile_pool(name="dram", bufs=2, space="DRAM"))
            late_ctx = ExitStack()
            late = late_ctx.enter_context(tc.tile_pool(name="late", bufs=1))
            qkv_ctx = ExitStack()
            qkv = qkv_ctx.enter_context(tc.tile_pool(name="qkv", bufs=1))

            # PSUM: "s" [128,2,512] x3 = 6 banks (scores; proj/fc borrow these
            # slots), "c" [65,512] x2 = 2 banks (ctx accumulators).
            def ps_s():
                return pat.tile([128, G, 512], f32, tag="s", name="ps_s", bufs=3)

            def ps_c():
                return pat.tile([DK + 1, 512], f32, tag="c", name="ps_c", bufs=2)

            def ps_f(n=512):
                return pat.tile([128, n], f32, tag="s", name="ps_f", bufs=3)

            # ---- input loads: weights first, then per-slab slices ------------
            wq_sb = qkv.tile([128, KC, CSL], bf16, tag="wq", name="wq")
            wk_sb = qkv.tile([128, KC, CSL], bf16, tag="wk", name="wk")
            wv_sb = qkv.tile([128, KC, CSL], bf16, tag="wv", name="wv")
            nc.sync.dma_start(out=wq_sb, in_=wq)
            nc.sync.dma_start(out=wk_sb, in_=wk)

            qT_sb = qkv.tile([128, KC, N], bf16, tag="qT", name="qT")
            kT_sb = qkv.tile([128, KC, N], bf16, tag="kT", name="kT")
            vT_sb = qkv.tile([128, KC, N], bf16, tag="vT", name="vT")
            for half in range(2):
                sl = slice(half * 1024, (half + 1) * 1024)
                for kc in range(KC):
                    nc.sync.dma_start(out=qT_sb[:, kc, sl],
                                      in_=qT[kc * 128:(kc + 1) * 128, sl])
                    nc.sync.dma_start(out=kT_sb[:, kc, sl],
                                      in_=kT[kc * 128:(kc + 1) * 128, sl])
            nc.sync.dma_start(out=wv_sb, in_=wv)
            for half in range(2):
                sl = slice(half * 1024, (half + 1) * 1024)
                for kc in range(KC):
                    nc.sync.dma_start(out=vT_sb[:, kc, sl],
                                      in_=vT[kc * 128:(kc + 1) * 128, sl])

            # late inputs on the scalar HWDGE ring so they don't delay the above
            wfc_sb = late.tile([128, CSL // 128, D], fp8, tag="wfc", name="wfc")
            nc.scalar.dma_start(out=wfc_sb, in_=wfc8)
            qres_sb = late.tile([128, ST, D], f32, tag="qres", name="qres")
            nc.scalar.dma_start(out=qres_sb, in_=qres)
            gamma_sb = late.tile([128, D], f32, tag="gamma", name="gamma")
            nc.scalar.dma_start(out=gamma_sb,
                                in_=bass.AP(tensor=gamma.tensor, offset=gamma.offset,
                                            ap=[[0, 128]] + gamma.ap))
            beta_sb = late.tile([128, D], f32, tag="beta", name="beta")
            nc.scalar.dma_start(out=beta_sb,
                                in_=bass.AP(tensor=beta.tensor, offset=beta.offset,
                                            ap=[[0, 128]] + beta.ap))
            xacc = qres_sb  # relu+residual accumulates in place over the residual

            # ---- projections -------------------------------------------------
            # qhT/khT: [dk, seq] per head, heads 2p / 2p+1 stacked on partition
            # halves.  k copies go through the scalar engine pre-attention so
            # DVE and ACT split the psum-evacuation work.
            qhT = [persist.tile([128, N], bf16, tag=f"qhT{p}", name=f"qhT{p}") for p in range(2)]
            khT = [persist.tile([128, N], bf16, tag=f"khT{p}", name=f"khT{p}") for p in range(2)]

            def qk_proj_pair(p, st, use_act=False):
                # q and k chains interleaved on two psum banks: consecutive
                # MMs never hit the same bank (WAW chain penalty) and LDW
                # overlaps the previous matmul.
                sl = slice(st * 512, (st + 1) * 512)
                psq = ps_f()
                psk = ps_f()
                for kc in range(KC):
                    nc.tensor.matmul(
                        psq, wq_sb[:, kc, p * 128:(p + 1) * 128],
                        qT_sb[:, kc, sl],
                        start=(kc == 0), stop=(kc == KC - 1))
                    nc.tensor.matmul(
                        psk, wk_sb[:, kc, p * 128:(p + 1) * 128],
                        kT_sb[:, kc, sl],
                        start=(kc == 0), stop=(kc == KC - 1))
                nc.vector.tensor_copy(out=qhT[p][:, sl], in_=psq)
                if use_act:
                    nc.scalar.copy(out=khT[p][:, sl], in_=psk)
                else:
                    nc.vector.tensor_copy(out=khT[p][:, sl], in_=psk)

            # vh pairs for DoubleRow ctx: [128 keys, 2 chunks, HL heads, 80]
            # (65 used: 64 dims + ones column; 80 keeps the DR pair stride
            # 16B-aligned).
            vhp = [persist.tile([128, G, HL, 80], fp8, tag=f"vhp{g}", name=f"vhp{g}")
                   for g in range(NG)]
            for g in range(NG):
                nc.vector.memset(vhp[g][:, :, :, DK:DK + 1], 1.0)

            def v_proj_pair(g):
                # chunks 2g / 2g+1 interleaved on two psum banks
                sc0 = 2 * g
                psa = ps_f(CSL)
                psb = ps_f(CSL)
                for kc in range(KC):
                    nc.tensor.matmul(
                        psa, vT_sb[:, kc, sc0 * 128:(sc0 + 1) * 128],
                        wv_sb[:, kc, :],
                        start=(kc == 0), stop=(kc == KC - 1))
                    nc.tensor.matmul(
                        psb, vT_sb[:, kc, (sc0 + 1) * 128:(sc0 + 2) * 128],
                        wv_sb[:, kc, :],
                        start=(kc == 0), stop=(kc == KC - 1))
                nc.vector.tensor_copy(
                    out=vhp[g][:, 0, :, 0:DK],
                    in_=psa.rearrange("p (h d) -> p h d", h=HL))
                nc.vector.tensor_copy(
                    out=vhp[g][:, 1, :, 0:DK],
                    in_=psb.rearrange("p (h d) -> p h d", h=HL))

            # ctxn: normalized context, fp8, [128 c, 2 cc-chunks, seq] --
            # cc chunk index == p (heads 2p, 2p+1 on partition halves).
            ctxn = persist.tile([128, 2, N], fp8, tag="ctxn", name="ctxn")
            ones_sb = persist.tile([1, DK], bf16, tag="ones", name="ones")
            nc.vector.memset(ones_sb, 1.0)

            def attention(p, t, extra=None):
                pc = [ps_c() for _ in range(2)]
                pse_prev = [None, None]

                def ctx_mm(g, s):
                    nc.tensor.matmul(
                        pc[s],
                        vhp[g][:, :, 2 * p + s, 0:DK + 1],
                        pse_prev[s],
                        start=(g == 0), stop=(g == NG - 1),
                        perf_mode=DR)

                for g in range(NG):
                    ppss = [ps_s(), ps_s()]
                    # scores: alternate row groups every MM so LDW overlaps
                    for j in range(G):
                        kc = g * G + j
                        for s in range(2):
                            lo = 64 * s
                            nc.tensor.matmul(
                                ppss[s][:, j, :],
                                khT[p][lo:lo + 64, kc * 128:(kc + 1) * 128],
                                qhT[p][lo:lo + 64, t * 512:(t + 1) * 512],
                                start=True, stop=True,
                                tile_position=(lo, 0))
                    # ctx for the previous group (exp ran during these scores)
                    if g > 0:
                        for s in range(2):
                            ctx_mm(g - 1, s)
                    if extra is not None:
                        extra(g)
                    for s in range(2):
                        pse = epool.tile([128, G, 512], fp8, tag="e", name="e")
                        if g in FAST_EXP_GROUPS:
                            # Schraudolph exp on DVE (bit-trick) + gpsimd
                            # bitcast-convert: offloads the ACT bottleneck.
                            u = epool.tile([128, G, 512], mybir.dt.int32,
                                           tag="u", name="u")
                            nc.vector.tensor_scalar(
                                out=u, in0=ppss[s], scalar1=EXP_A, scalar2=EXP_B,
                                op0=Alu.mult, op1=Alu.add)
                            nc.gpsimd.tensor_copy(out=pse, in_=u.bitcast(f32))
                        else:
                            nc.scalar.activation(out=pse, in_=ppss[s], func=AF.Exp,
                                                 scale=1.0 / float(np.sqrt(DK)))
                        pse_prev[s] = pse
                for s in range(2):
                    ctx_mm(NG - 1, s)

                # normalize: broadcast the ones-row (softmax denominator)
                # across 64 partitions with a K=1 ones-matmul on the PE, take
                # the fast approx reciprocal (1.2 cpe vs 6), multiply into the
                # fp8 ctxn tile.  No gpsimd / DRAM round trip on this path.
                for s in range(2):
                    d1 = work.tile([1, 512], bf16, tag="rb", name="rb")
                    nc.vector.tensor_copy(out=d1, in_=pc[s][DK:DK + 1, :])
                    ps_rb = ps_f()
                    nc.tensor.matmul(ps_rb[0:DK, :], ones_sb, d1,
                                     start=True, stop=True)
                    rb = work.tile([DK, 512], f32, tag="rb64", name="rb64")
                    nc.vector.reciprocal_approx_fast(out=rb, in_=ps_rb[0:DK, :])
                    nc.vector.tensor_mul(
                        out=ctxn[64 * s:64 * (s + 1), p, t * 512:(t + 1) * 512],
                        in0=pc[s][0:DK, :], in1=rb)

            # fc (fp8 DoubleRow, both 128-chunks contracted per MM) + chunked
            # ReduceScatter per slab; each core ends with rows
            # [t*512 + rank*128, +128) of its batch.
            def fc_mm(t, qq, nh):
                qc = t * 4 + qq
                ps = ps_f()
                nc.tensor.matmul(
                    ps,
                    ctxn[:, :, qc * 128:(qc + 1) * 128],
                    wfc_sb[:, :, nh * 512:(nh + 1) * 512],
                    start=True, stop=True, perf_mode=DR)
                fcs = work.tile([128, 512], fp8, tag="fcs", name="fcs", bufs=8)
                nc.vector.tensor_scalar(out=fcs, in0=ps,
                                        scalar1=1.0 / WFC_SCALE, scalar2=None,
                                        op0=Alu.mult)
                return fcs

            rs_outs = []

            def fc_rs(t, fcs_parts):
                rs_in = dram.tile([512, D], fp8, tag="rs_in", name="rs_in",
                                  bufs=4)
                rs_out = dram.tile([128, D], fp8, tag="rs_out", name="rs_out",
                                   bufs=4)
                for (qq, nh), fcs in fcs_parts.items():
                    nc.sync.dma_start(
                        out=rs_in[qq * 128:(qq + 1) * 128, nh * 512:(nh + 1) * 512],
                        in_=fcs)
                nc.gpsimd.collective_compute(
                    "ReduceScatter",
                    mybir.AluOpType.add,
                    replica_groups=[[0, 1, 2, 3], [4, 5, 6, 7]],
                    ins=[rs_in.opt()],
                    outs=[rs_out.opt()])
                rs_outs.append(rs_out)

            # ---- schedule ----------------------------------------------------
            for st in range(ST):
                qk_proj_pair(0, st, use_act=True)
            v_proj_pair(0)
            v_proj_pair(1)

            # A(0,0) group g consumes vhp[g]; emit vhp[g+2] at group g, then
            # start qk_proj(1) in the last two group slots.
            def extra00(g):
                if g + 2 < NG:
                    v_proj_pair(g + 2)
                elif g == 6:
                    qk_proj_pair(1, 0)
                elif g == 7:
                    qk_proj_pair(1, 1)
            attention(0, 0, extra=extra00)
            qk_proj_pair(1, 2)
            qk_proj_pair(1, 3)
            attention(1, 0)

            # fc for slab t-1: first half dense right after A(1,t-1) (early
            # collective trigger), second half spread into A(0,t) as PE
            # fillers for the ACT-paced attention groups.
            fcs_pending = {}

            def extra_fc(t_prev):
                def fill(g):
                    if g < 4:
                        qq, nh = 2 + g // 2, g % 2
                        fcs_pending[(qq, nh)] = fc_mm(t_prev, qq, nh)
                return fill

            for t in range(1, ST):
                for qq in range(2):
                    for nh in range(2):
                        fcs_pending[(qq, nh)] = fc_mm(t - 1, qq, nh)
                attention(0, t, extra=extra_fc(t - 1))
                fc_rs(t - 1, fcs_pending)
                fcs_pending = {}
                attention(1, t)
            for qq in range(4):
                for nh in range(2):
                    fcs_pending[(qq, nh)] = fc_mm(ST - 1, qq, nh)
            fc_rs(ST - 1, fcs_pending)
            qkv_ctx.close()

            # ---- tail: rs loads + relu/residual + layernorm ------------------
            # rstd on the DVE (bit-trick seed + 2 Newton iterations) keeps the
            # scalar engine's ACT table on the exp set for the whole kernel.
            i32 = mybir.dt.int32
            MAGIC = float(0x5F3759DF)

            for t in range(ST):
                rs_sb = work.tile([128, D], fp8, tag="rs_sb", name="rs_sb")
                # corner-write gate: forces the scheduler to keep this load
                # (which waits on the collective) after the last attention
                # tile, so it can't hoist into mid-kernel engine FIFOs.
                nc.gpsimd.tensor_copy(out=rs_sb[0:1, 0:1],
                                      in_=ctxn[0:1, 1, N - 1:N])
                nc.gpsimd.dma_start(out=rs_sb, in_=rs_outs[t])
                nc.vector.scalar_tensor_tensor(
                    out=xacc[:, t, :], in0=rs_sb, scalar=0.0,
                    in1=qres_sb[:, t, :], op0=Alu.max, op1=Alu.add)
                x = xacc[:, t, :]
                stats = work.tile([128, 2, 6], f32, tag="stats", name="stats")
                nc.vector.bn_stats(out=stats[:, 0, :], in_=x[:, 0:512])
                nc.vector.bn_stats(out=stats[:, 1, :], in_=x[:, 512:1024])
                mv = work.tile([128, 2], f32, tag="mv", name="mv")
                nc.vector.bn_aggr(out=mv, in_=stats)
                v = work.tile([128, 1], f32, tag="veps", name="veps")
                nc.vector.tensor_scalar(out=v, in0=mv[:, 1:2], scalar1=LN_EPS,
                                        scalar2=None, op0=Alu.add)
                # rsqrt: y0 = bitcast(0x5F3759DF - bits(v)/2) via f32 arithmetic
                # on the bit pattern (exact enough for a seed), then 2x Newton.
                si = work.tile([128, 1], i32, tag="rss", name="rss")
                nc.vector.tensor_scalar(out=si, in0=v.bitcast(i32), scalar1=-0.5,
                                        scalar2=MAGIC, op0=Alu.mult, op1=Alu.add)
                yk = si.bitcast(f32)
                for _ in range(2):
                    a = work.tile([128, 1], f32, tag="rsa", name="rsa")
                    nc.vector.tensor_mul(out=a, in0=yk, in1=yk)
                    nc.vector.tensor_mul(out=a, in0=a, in1=v)
                    nc.vector.tensor_scalar(out=a, in0=a, scalar1=-0.5,
                                            scalar2=1.5, op0=Alu.mult, op1=Alu.add)
                    yn = work.tile([128, 1], f32, tag="rsy", name="rsy")
                    nc.vector.tensor_mul(out=yn, in0=yk, in1=a)
                    yk = yn
                xo = work.tile([128, D], f32, tag="xo", name="xo")
                nc.vector.tensor_scalar(out=xo, in0=x,
                                        scalar1=mv[:, 0:1], scalar2=yk,
                                        op0=Alu.subtract, op1=Alu.mult)
                nc.vector.tensor_mul(out=xo, in0=xo, in1=gamma_sb)
                nc.vector.tensor_add(out=xo, in0=xo, in1=beta_sb)
                nc.sync.dma_start(out=y[t * 128:(t + 1) * 128, :], in_=xo)
            late_ctx.close()

    nc.compile()
    return nc


def kernel(q, k, v, w_qs, w_ks, w_vs, w_fc, ln_gamma, ln_beta):
    from concourse import bass_utils

    if "nc" not in _CACHE:
        _CACHE["nc"] = _build()
    nc = _CACHE["nc"]

    bf = ml_dtypes.bfloat16
    f8 = ml_dtypes.float8_e4m3
    q = np.asarray(q, np.float32)
    k = np.asarray(k, np.float32)
    v = np.asarray(v, np.float32)
    w_fc = np.asarray(w_fc, np.float32)

    def warr(w, cs):
        # [D, CSL] -> SBUF layout [128, KC, CSL]
        a = np.asarray(w, np.float32)[:, cs].reshape(D // 128, 128, CSL)
        return np.ascontiguousarray(a.transpose(1, 0, 2)).astype(bf)

    in_maps = []
    for i in range(N_CORES):
        bi, hg = i // 4, i % 4
        cs = slice(hg * CSL, (hg + 1) * CSL)
        row_idx = np.concatenate(
            [np.arange(t * 512 + hg * 128, t * 512 + (hg + 1) * 128) for t in range(4)])
        wfc_a = (w_fc[cs, :] * WFC_SCALE).reshape(CSL // 128, 128, D)
        qres_a = q[bi][row_idx].reshape(4, 128, D)
        in_maps.append({
            "qT": np.ascontiguousarray(q[bi].T).astype(bf),
            "kT": np.ascontiguousarray(k[bi].T).astype(bf),
            "vT": np.ascontiguousarray(v[bi].T).astype(bf),
            "wq": warr(w_qs, cs),
            "wk": warr(w_ks, cs),
            "wv": warr(w_vs, cs),
            "wfc8": np.ascontiguousarray(wfc_a.transpose(1, 0, 2)).astype(f8),
            "qres": np.ascontiguousarray(qres_a.transpose(1, 0, 2)).astype(np.float32),
            "gamma": np.ascontiguousarray(np.asarray(ln_gamma, np.float32)),
            "beta": np.ascontiguousarray(np.asarray(ln_beta, np.float32)),
        })

    run_kwargs = dict(_CACHE.get("run_kwargs", {}))
    res = bass_utils.run_bass_kernel_spmd(nc, in_maps, core_ids=list(range(N_CORES)),
                                          **run_kwargs)
    _CACHE["last_res"] = res
    out = np.empty((B, N, D), np.float32)
    for i in range(N_CORES):
        bi, hg = i // 4, i % 4
        yi = res.results[i]["y"]
        for t in range(4):
            out[bi, t * 512 + hg * 128:t * 512 + (hg + 1) * 128, :] = \
                yi[t * 128:(t + 1) * 128, :]
    return out
